# revision 1
# baseline (speedup 1.0000x reference)
"""Multi-head attention (B=4, N=1568, C=768, H=12) on 8 TRN2 NeuronCores.

Sharding: query-parallel. Core c handles batch b = c // 2 and query half
half = c % 2 (784 query tokens). Each core computes K/V projections for the
full 1568 tokens of its batch (duplicated across the pair), Q projection
for its 784 tokens, full attention for all 12 heads over its queries, and
the output projection. No cross-core communication.

Host-side tricks:
  - tokens are rotated per core so its own query half sits at columns 0:784
    of xT; the key order is then a (core-dependent) permutation, which
    softmax attention is invariant to. This removes the separate xqT input.
  - v_bias is folded into the projection bias:
      out = (attn + 1 (x) v_bias) @ proj_w + proj_b
          = attn @ proj_w + (proj_b + v_bias @ proj_w)
  - the softmax 1/sqrt(D) scale is folded into the exp activation's scale.

Device layouts (per core):
  xT   [768, 1568]  x[b].T rotated       (feature-major)
  K^T  [768, 1568]  feature-major K
  Q^T  [768, 784]   feature-major Q (+q_bias)
  V    [1568, 780]  token-major V as 12 heads x (64 cols + ones col)
  scores^T [keys, q] per (head, key-tile) in PSUM -> exp -> bf16 SBUF
  PV   -> psum [65, 784]: rows 0-63 = (expS @ V)^T, row 64 = softmax denom
  attn^T [768, 784]  normalized attention, feature-major
  outT [768, 784]   proj_w.T @ attn^T + (proj_b + v_bias @ proj_w)
"""

import numpy as np
import ml_dtypes

B, N, C = 4, 1568, 768
H = 12
D = 64
NQ = N // 2          # 784 queries per core
SCALE = D ** -0.5
N_CORES = 8
KT = [128] * 12 + [32]          # key tiles (sum = 1568)
QCH = [(0, 512), (512, 272)]    # query chunks (PSUM-bank aligned)
TCH = [(0, 392), (392, 392), (784, 392), (1176, 392)]  # token chunks (phase A)

_cache = {}


def _build_program():
    import concourse.mybir as mybir
    from concourse import bacc
    from concourse.tile import TileContext

    f32 = mybir.dt.float32
    f32r = mybir.dt.float32r
    bf16 = mybir.dt.bfloat16
    Exp = mybir.ActivationFunctionType.Exp

    nc = bacc.Bacc("TRN2", target_bir_lowering=False, debug=False,
                   num_devices=N_CORES)

    xT_d = nc.dram_tensor("xT", [C, N], bf16, kind="ExternalInput")
    wqk_d = nc.dram_tensor("wqk", [C, 2 * C], bf16, kind="ExternalInput")
    wv_d = nc.dram_tensor("wv", [C, C], bf16, kind="ExternalInput")
    wp_d = nc.dram_tensor("wproj", [C, C], f32r, kind="ExternalInput")
    qb_d = nc.dram_tensor("qb", [128, 6], f32, kind="ExternalInput")
    pb_d = nc.dram_tensor("pb", [128, 6], f32, kind="ExternalInput")
    out_d = nc.dram_tensor("outT", [C, NQ], f32, kind="ExternalOutput")

    with TileContext(nc) as tc:
        persist_cm = tc.tile_pool(name="persist", bufs=1)
        persist = persist_cm.__enter__()
        kT = [persist.tile([128, N], bf16, tag=f"kT{j}", name=f"kT{j}")
              for j in range(6)]
        qT = [persist.tile([128, NQ], bf16, tag=f"qT{j}", name=f"qT{j}")
              for j in range(6)]
        v_sb = [persist.tile([128, H * (D + 1)], bf16, tag=f"v{t}", name=f"v{t}")
                for t in range(13)]
        attn = [persist.tile([128, NQ], f32r, tag=f"at{j}", name=f"at{j}")
                for j in range(6)]
        qb_sb = persist.tile([128, 6], f32, tag="qb")
        pb_sb = persist.tile([128, 6], f32, tag="pb")
        nc.sync.dma_start(out=qb_sb, in_=qb_d[:])
        nc.sync.dma_start(out=pb_sb, in_=pb_d[:])

        wpp_cm = tc.tile_pool(name="wpp", bufs=1)
        wpp = wpp_cm.__enter__()
        wp_sb = [wpp.tile([128, C], f32r, tag=f"wp{j}", name=f"wp{j}")
                 for j in range(6)]

        # ========== phases A+B merged: QKV projections + attention ==========
        # One PSUM layout for both: psA (1 bank x 2) for projections,
        # psS (2 banks x 2) for scores, psO (2 banks x 1) for PV accum.
        phA_cm = tc.tile_pool(name="phA", bufs=1)
        phA = phA_cm.__enter__()
        xT = [phA.tile([128, N], bf16, tag=f"xT{j}", name=f"xTs{j}")
              for j in range(6)]
        wqk = [phA.tile([128, 2 * C], bf16, tag=f"wqk{j}", name=f"wqks{j}")
               for j in range(6)]
        wv = [phA.tile([128, C], bf16, tag=f"wv{j}", name=f"wvs{j}")
              for j in range(6)]
        for j in range(6):
            nc.sync.dma_start(out=xT[j], in_=xT_d[j * 128:(j + 1) * 128, :])
            nc.sync.dma_start(out=wqk[j][:, C:2 * C],
                              in_=wqk_d[j * 128:(j + 1) * 128, C:2 * C])
        for j in range(6):
            nc.sync.dma_start(out=wqk[j][:, 0:C],
                              in_=wqk_d[j * 128:(j + 1) * 128, 0:C])
            nc.sync.dma_start(out=wv[j], in_=wv_d[j * 128:(j + 1) * 128, :])

        psA_cm = tc.tile_pool(name="psA", bufs=2, space="PSUM")
        psA = psA_cm.__enter__()
        psS_cm = tc.tile_pool(name="psS", bufs=2, space="PSUM")
        psS = psS_cm.__enter__()
        psO_cm = tc.tile_pool(name="psO", bufs=1, space="PSUM")
        psO = psO_cm.__enter__()
        phB_cm = tc.tile_pool(name="phB", bufs=3)
        phB = phB_cm.__enter__()
        phBn_cm = tc.tile_pool(name="phBn", bufs=2)
        phBn = phBn_cm.__enter__()

        def emit_k(ft):
            for (t0, tw) in TCH:
                ps = psA.tile([128, 512], f32, tag="psA", name=f"k{ft}_{t0}")
                for j in range(6):
                    nc.tensor.matmul(
                        ps[:, 0:tw],
                        wqk[j][:, C + ft * 128:C + ft * 128 + 128],
                        xT[j][:, t0:t0 + tw],
                        start=(j == 0), stop=(j == 5),
                    )
                nc.vector.tensor_copy(kT[ft][:, t0:t0 + tw], ps[:, 0:tw])

        def emit_q(ft):
            for (t0, tw) in TCH[:2]:
                ps = psA.tile([128, 512], f32, tag="psA", name=f"q{ft}_{t0}")
                for j in range(6):
                    nc.tensor.matmul(
                        ps[:, 0:tw],
                        wqk[j][:, ft * 128:ft * 128 + 128],
                        xT[j][:, t0:t0 + tw],
                        start=(j == 0), stop=(j == 5),
                    )
                nc.vector.tensor_scalar(
                    out=qT[ft][:, t0:t0 + tw], in0=ps[:, 0:tw],
                    scalar1=qb_sb[:, ft:ft + 1], scalar2=None,
                    op0=mybir.AluOpType.add,
                )

        def emit_v(tt):
            mt = KT[tt]
            v3 = v_sb[tt].rearrange("p (h e) -> p h e", h=H)
            for vch in range(2):
                ps = psA.tile([128, 512], f32, tag="psA", name=f"v{tt}_{vch}")
                for j in range(6):
                    nc.tensor.matmul(
                        ps[0:mt, 0:384],
                        xT[j][:, tt * 128:tt * 128 + mt],
                        wv[j][:, vch * 384:(vch + 1) * 384],
                        start=(j == 0), stop=(j == 5),
                    )
                nc.vector.tensor_copy(
                    v3[0:mt, vch * 6:(vch + 1) * 6, 0:64],
                    ps[0:mt, 0:384].rearrange("p (h e) -> p h e", h=6),
                )
            nc.vector.memset(v3[0:mt, :, 64:65], 1.0)

        with nc.named_scope("qkv"):
            emit_k(0)
            emit_q(0)

        with nc.named_scope("attn"):
            po_of = {}

            def emit_qk(h, tt):
                ft, fo = h // 2, (h % 2) * 64
                mt = KT[tt]
                ps = psS.tile([128, 1024], f32, tag="psS", name=f"s{h}_{tt}")
                for (q0, qw) in QCH:
                    nc.tensor.matmul(
                        ps[0:mt, q0:q0 + qw],
                        kT[ft][fo:fo + 64, tt * 128:tt * 128 + mt],
                        qT[ft][fo:fo + 64, q0:q0 + qw],
                        start=True, stop=True,
                    )
                return ps

            def emit_exp_pv(h, tt, ps):
                mt = KT[tt]
                if tt == 0:
                    po_of[h] = psO.tile([65, 1024], f32, tag="psO",
                                        name=f"po{h}")
                po = po_of[h]
                ex = phB.tile([128, NQ], bf16, tag="ex", name=f"ex{h}_{tt}")
                nc.scalar.activation(out=ex[0:mt, :], in_=ps[0:mt, 0:NQ],
                                     func=Exp, scale=SCALE)
                vh = v_sb[tt].rearrange("p (h e) -> p h e", h=H)[0:mt, h, :]
                for (q0, qw) in QCH:
                    nc.tensor.matmul(
                        po[:, q0:q0 + qw],
                        vh,
                        ex[0:mt, q0:q0 + qw],
                        start=(tt == 0), stop=(tt == 12),
                    )
                if tt == 12:
                    emit_normalize(h, po)

            def emit_normalize(h, po):
                # rows 0-63 / row 64 (denominator). Custom DVE / gpsimd ops
                # only work from partition 0, so evict PSUM to SBUF,
                # DMA-shift the denominator row to partition 0, then
                # recip+broadcast+multiply there.
                ft, fo = h // 2, (h % 2) * 64
                t65 = phBn.tile([65, NQ], f32, tag="t65", name=f"t65_{h}")
                nc.vector.tensor_copy(t65, po[:, 0:NQ])
                rec0 = phBn.tile([1, NQ], f32, tag="rec0", name=f"rc0_{h}")
                nc.gpsimd.dma_start(out=rec0, in_=t65[64:65, :])
                rec1 = phBn.tile([1, NQ], f32, tag="rec1", name=f"rc1_{h}")
                nc.vector.reciprocal_approx_fast(out=rec1, in_=rec0)
                rb = phBn.tile([64, NQ], f32, tag="rb", name=f"rb_{h}")
                nc.gpsimd.partition_broadcast(rb, rec1)
                stage = phBn.tile([64, NQ], f32r, tag="stage", name=f"st_{h}")
                nc.vector.tensor_mul(stage, t65[0:64, :], rb)
                nc.gpsimd.dma_start(out=attn[ft][fo:fo + 64, :], in_=stage)

            pend = None
            for h in range(H):
                for tt in range(13):
                    if h == 0:
                        emit_v(tt)          # V tiles stream in under head 0
                    ps = emit_qk(h, tt)
                    if pend is not None:
                        emit_exp_pv(*pend)
                    pend = (h, tt, ps)
                # interleave remaining K/Q projection blocks and the
                # wproj load into the attention stream (PE gap filler);
                # emitted at the end of odd heads, where the PE has slack
                if h in (1, 3, 5, 7, 9):
                    emit_k(h // 2 + 1)
                    emit_q(h // 2 + 1)
                if h == 8:
                    for j in range(6):
                        nc.sync.dma_start(
                            out=wp_sb[j],
                            in_=wp_d[j * 128:(j + 1) * 128, :])
            emit_exp_pv(*pend)

        phBn_cm.__exit__(None, None, None)
        phB_cm.__exit__(None, None, None)
        psO_cm.__exit__(None, None, None)
        psS_cm.__exit__(None, None, None)
        psA_cm.__exit__(None, None, None)
        phA_cm.__exit__(None, None, None)

        # ================= phase C: output projection =================
        with (
            nc.named_scope("proj"),
            tc.tile_pool(name="psP", bufs=4, space="PSUM") as psP,
            tc.tile_pool(name="phC", bufs=3) as phC,
        ):
            for ot in range(6):
                for (q0, qw) in QCH:
                    ps = psP.tile([128, 512], f32, tag="psP")
                    for j in range(6):
                        nc.tensor.matmul(
                            ps[:, 0:qw],
                            wp_sb[j][:, ot * 128:(ot + 1) * 128],
                            attn[j][:, q0:q0 + qw],
                            start=(j == 0), stop=(j == 5),
                        )
                    ob = phC.tile([128, 512], f32, tag="ob")
                    nc.vector.tensor_scalar(
                        out=ob[:, 0:qw], in0=ps[:, 0:qw],
                        scalar1=pb_sb[:, ot:ot + 1], scalar2=None,
                        op0=mybir.AluOpType.add,
                    )
                    nc.sync.dma_start(
                        out=out_d[ot * 128:(ot + 1) * 128, q0:q0 + qw],
                        in_=ob[:, 0:qw])

        wpp_cm.__exit__(None, None, None)
        persist_cm.__exit__(None, None, None)

    nc.compile()
    return nc


def _get_program():
    if "nc" not in _cache:
        _cache["nc"] = _build_program()
    return _cache["nc"]


def _make_in_maps(x, qkv_w, q_bias, v_bias, proj_w, proj_b):
    wqk = np.ascontiguousarray(qkv_w[:, :2 * C])      # [C, 2C] (q cols, k cols)
    wv = np.ascontiguousarray(qkv_w[:, 2 * C:])       # [C, C]
    qb = np.zeros((128, 6), np.float32)
    qb[:, :] = q_bias.reshape(6, 128).T
    pb_eff = proj_b + v_bias @ proj_w                  # fold v_bias into proj
    pb = np.zeros((128, 6), np.float32)
    pb[:, :] = pb_eff.reshape(6, 128).T

    in_maps = []
    for c in range(N_CORES):
        b, half = c // 2, c % 2
        # rotate tokens so this core's query half sits at columns 0:NQ;
        # key order becomes a permutation, which softmax attention is
        # invariant to
        xT = np.ascontiguousarray(
            np.roll(x[b].T, -half * NQ, axis=1)).astype(ml_dtypes.bfloat16)
        in_maps.append({
            "xT": xT, "wqk": wqk.astype(ml_dtypes.bfloat16),
            "wv": wv.astype(ml_dtypes.bfloat16),
            "wproj": proj_w, "qb": qb, "pb": pb,
        })
    return in_maps


def kernel(x, qkv_w, q_bias, v_bias, proj_w, proj_b):
    from concourse.bass_utils import run_bass_kernel_spmd

    x = np.asarray(x, dtype=np.float32)
    qkv_w = np.asarray(qkv_w, dtype=np.float32)
    q_bias = np.asarray(q_bias, dtype=np.float32)
    v_bias = np.asarray(v_bias, dtype=np.float32)
    proj_w = np.asarray(proj_w, dtype=np.float32)
    proj_b = np.asarray(proj_b, dtype=np.float32)

    nc = _get_program()
    in_maps = _make_in_maps(x, qkv_w, q_bias, v_bias, proj_w, proj_b)
    _cache["in_maps"] = in_maps

    res = run_bass_kernel_spmd(nc, in_maps, list(range(N_CORES)))
    out = np.empty((B, N, C), np.float32)
    for c in range(N_CORES):
        b, half = c // 2, c % 2
        out[b, half * NQ:(half + 1) * NQ, :] = res.results[c]["outT"].T
    return out

